# revision 32
# baseline (speedup 1.0000x reference)
"""Conv1D-MHSA (sketched linear attention) Trainium2 kernel.

Math: the reference computes, per (batch b, head h):
    q = conv1d_K3(x_pad, q_w) + q_b ; k likewise ; v = conv1d_K1(x, v_w)
    phi_q = sqrt(R) * tanh((q^T g1_q) * (q^T g2_q) / sqrt(R))  (phi_k likewise)
    scores = phi_q @ phi_k^T                     [L, L]
    o = (scores / (rowsum(scores) + 1e-6)) @ v   [L, D]
    out = concat_h(o) @ proj_w^T + proj_b
There is no softmax, so the L x L matrix is never needed:
    o = diag(1/(phi_q @ s_k)) . phi_q @ (phi_k^T v),   s_k = colsum(phi_k)
(The sqrt(R) scales cancel; eps is ~5e5x below min |den| and is dropped.
The projection commutes with the row division: project first, divide last.)

Precision scheme (measured): the denominator path amplifies operand rounding
~4000x (near-zero rows of den), so everything feeding phi_q / s_k needs
fp32-level operands.  Strict fp32 matmuls cost 4 cycles/row on the PE, but
fp32r costs 1 cycle/row (free dim >= 256) at ~13-bit operand precision.  So
sensitive operands are split EXACTLY as hi (12-bit mantissa, survives f32r
storage unchanged) + lo (fp32 residual, f32r-stored), and each sensitive
matmul runs 3 fp32r passes (hi*hi + hi*lo + lo*hi, PSUM-accumulated):
3 cycles/row with ~2^-25 composite error.  Weights/x/g split on the host;
q/k and phi_q split on device (ACT copy rounds to the f32r grid, Pool
subtracts the residual).  M~ stays strict fp32 (129-wide rhs would not hit
the fast f32r rate anyway).  The numerator/projection path is tolerant and
runs single-pass f32r (better than the old bf16).  v-conv stays bf16.

Sharding: head-parallel over 8 cores (head h -> core h, both batches).  Each
core returns a partial projection in [j, l] layout [B, D, L]; the host sums
the 8 partials, transposes to [B, L, D] and adds proj_b.  gamma/beta affine
and conv biases are folded into weights on the host.
"""

import numpy as np
from contextlib import ExitStack

import concourse.bacc as bacc
import concourse.mybir as mybir
import concourse.tile as tile
from concourse.bass_utils import run_bass_kernel_spmd

F32 = mybir.dt.float32
F32R = mybir.dt.float32r
FP16 = mybir.dt.float16
BF16 = mybir.dt.bfloat16
AF = mybir.ActivationFunctionType

B = 2          # batch
D = 128        # per-head dim (= partition size)
L = 2048       # sequence length
H = 8          # heads == cores
R = 128        # sketch dim
KS = 3         # conv kernel size
LP = L + KS - 1
NCH = L // 512   # 4 big chunks
NT = L // 128    # 16 tiles
NG = 2           # m-tiles per k/v evacuation group (double-buffered PSUM)
SQRT_R = float(np.sqrt(R))

# fp16 blob: hi/lo split weights + x (free-dim offsets into [128, HBLOB_W])
HOFF_QKWH = 0                        # q/k conv weights hi [2, 3, 128] -> 768
HOFF_QKWL = HOFF_QKWH + 2 * KS * D   # lo -> 1536
HOFF_GH = HOFF_QKWL + 2 * KS * D     # sketch mats hi [4, 128] -> 2048
HOFF_GL = HOFF_GH + 4 * R            # lo -> 2560
HOFF_XH0 = HOFF_GL + 4 * R           # xpad batch 0 hi [2050]
HOFF_XL0 = HOFF_XH0 + LP             # lo
HW_W = HOFF_XL0 + LP                 # 6660 (end of batch-0 block)
HBLOB_W = HW_W + 2 * LP              # x1 hi/lo appended
# fp32 blob: [pw (128) | qk biases (2) | strict g (4x128)]
OFF_PW = 0
OFF_QKB = OFF_PW + D
OFF_G = OFF_QKB + 2
BLOB_W = OFF_G + 4 * R
# bf16 blob: [vw (128) | x0 (2050) | x1 (2050)] = 4228
BOFF_VW = 0
BOFF_X0 = BOFF_VW + D
BBLOB_W = BOFF_X0 + 2 * LP

_built_nc = None
last_results = None


def _build():
    nc = bacc.Bacc(None, target_bir_lowering=False)
    hblob_d = nc.declare_dram_parameter("hblob", [D, HBLOB_W], FP16, isOutput=False)
    blob_d = nc.declare_dram_parameter("blob", [D, BLOB_W], F32, isOutput=False)
    bblob_d = nc.declare_dram_parameter("bblob", [D, BBLOB_W], BF16, isOutput=False)
    out_d = nc.declare_dram_parameter("outp", [B, D, L], F32, isOutput=True)

    def rr(ap):
        return ap.bitcast(F32R)

    with ExitStack() as ctx:
        tc = ctx.enter_context(tile.TileContext(nc))
        consts = ctx.enter_context(tc.tile_pool(name="consts", bufs=1))
        perb = ctx.enter_context(tc.tile_pool(name="perb", bufs=2))
        work = ctx.enter_context(tc.tile_pool(name="work", bufs=3))
        small = ctx.enter_context(tc.tile_pool(name="small", bufs=2))
        psA = ctx.enter_context(tc.tile_pool(name="psA", bufs=4, space="PSUM"))
        psK = ctx.enter_context(tc.tile_pool(name="psK", bufs=2, space="PSUM"))
        psV = ctx.enter_context(tc.tile_pool(name="psV", bufs=1, space="PSUM"))
        psM = ctx.enter_context(tc.tile_pool(name="psM", bufs=1, space="PSUM"))

        # input DMAs spread over three rings: weights via SWDGE (lowest
        # first-byte latency), x0 hi/lo via the SP HWDGE ring, x1 + bf16
        # blob via the ACT HWDGE ring — conv can start early
        wth = consts.tile([D, HOFF_GH], FP16, tag="wth")
        nc.gpsimd.dma_start(out=wth[:, 0:HOFF_QKWL],
                            in_=hblob_d[:, 0:HOFF_QKWL])
        qkbt = consts.tile([D, 2], F32, tag="qkbt")
        nc.gpsimd.dma_start(out=qkbt, in_=blob_d[:, OFF_QKB : OFF_QKB + 2])
        nc.gpsimd.dma_start(out=wth[:, HOFF_QKWL:HOFF_GH],
                            in_=hblob_d[:, HOFF_QKWL:HOFF_GH])
        gt = consts.tile([D, 4 * R], F32, tag="gt")
        nc.gpsimd.dma_start(out=gt, in_=blob_d[:, OFF_G : OFF_G + 4 * R])
        pwt = consts.tile([D, D], F32R, tag="pwt")
        nc.gpsimd.dma_start(out=pwt, in_=rr(blob_d[:, OFF_PW : OFF_PW + D]))
        x0h = consts.tile([D, LP], FP16, tag="x0h")
        x0l = consts.tile([D, LP], FP16, tag="x0l")
        for s, e in ((0, 1025), (1025, LP)):
            nc.sync.dma_start(out=x0h[:, s:e],
                              in_=hblob_d[:, HOFF_XH0 + s : HOFF_XH0 + e])
            nc.sync.dma_start(out=x0l[:, s:e],
                              in_=hblob_d[:, HOFF_XL0 + s : HOFF_XL0 + e])
        x1h = consts.tile([D, LP], FP16, tag="x1h")
        x1l = consts.tile([D, LP], FP16, tag="x1l")
        nc.scalar.dma_start(out=x1h, in_=hblob_d[:, HW_W : HW_W + LP])
        nc.scalar.dma_start(out=x1l, in_=hblob_d[:, HW_W + LP : HW_W + 2 * LP])
        bb = consts.tile([D, BBLOB_W], BF16, tag="bb")
        nc.scalar.dma_start(out=bb, in_=bblob_d[:])

        qkwh = wth[:, HOFF_QKWH:HOFF_QKWL].rearrange("p (a t d) -> p a t d",
                                                     a=2, t=KS)
        qkwl = wth[:, HOFF_QKWL:HOFF_GH].rearrange("p (a t d) -> p a t d",
                                                   a=2, t=KS)
        g_s = gt.rearrange("p (a r) -> p a r", a=4)
        pw_f = pwt
        xh = [x0h, x1h]
        xl = [x0l, x1l]
        vw_b = bb[:, BOFF_VW : BOFF_VW + D]
        xb = [bb[:, BOFF_X0 + b * LP : BOFF_X0 + (b + 1) * LP] for b in range(B)]

        for b in range(B):
            phiq = perb.tile([R, L], F32, tag="phiq")
            phik = perb.tile([128, NT, R], F32, tag="phik")
            vau = perb.tile([128, NT, R + 1], F32, tag="vau")
            nc.vector.memset(vau[:, :, R], 1.0)
            g12k = g_s[:, 2:4, :].rearrange("p a r -> p (a r)")

            # ---- conv chunks (9-pass split f32r), then per-chunk sketches
            for p in range(2):
                for c in range(NCH):
                    ps = psA.tile([128, 512], F32, tag="psA")
                    mm = []
                    for t in range(KS):
                        xsl = slice(c * 512 + t, c * 512 + t + 512)
                        mm += [(qkwh[:, p, t, :], xh[b][:, xsl]),
                               (qkwl[:, p, t, :], xh[b][:, xsl])]
                    for t in range(KS):
                        xsl = slice(c * 512 + t, c * 512 + t + 512)
                        mm += [(qkwh[:, p, t, :], xl[b][:, xsl])]
                    for i, (w_, x_) in enumerate(mm):
                        nc.tensor.matmul(ps, lhsT=w_, rhs=x_, start=(i == 0),
                                         stop=(i == len(mm) - 1))
                    qc = work.tile([D, 512], F32, tag="qc")
                    nc.scalar.add(qc, ps, qkbt[:, p : p + 1])
                    if p == 0:
                        u1 = psA.tile([128, 512], F32, tag="psA")
                        u2 = psA.tile([128, 512], F32, tag="psA")
                        for ui, u in ((0, u1), (1, u2)):
                            nc.tensor.matmul(u, lhsT=g_s[:, ui, :], rhs=qc,
                                             start=True, stop=True)
                        u1s = work.tile([128, 512], F32, tag="u1s")
                        nc.vector.tensor_copy(u1s, u1)
                        nc.vector.tensor_mul(phiq[:, c * 512 : (c + 1) * 512],
                                             u1s, u2)
                    else:
                        for mg in range(512 // (NG * 128)):
                            uu = psK.tile([128, NG, 2 * R], F32, tag="uu")
                            vp = psV.tile([128, NG, R], F32, tag="vp")
                            for j in range(NG):
                                off = (mg * NG + j) * 128
                                mt = c * 4 + mg * NG + j
                                kl = qc[:, off : off + 128]
                                nc.tensor.matmul(uu[:, j, :], lhsT=kl,
                                                 rhs=g12k, start=True, stop=True)
                                nc.tensor.matmul(
                                    vp[:, j, :],
                                    lhsT=xb[b][:, KS - 1 + mt * 128 :
                                               KS - 1 + (mt + 1) * 128],
                                    rhs=vw_b, start=True, stop=True)
                            sl = slice(c * 4 + mg * NG, c * 4 + (mg + 1) * NG)
                            u1ks = work.tile([128, NG, R], F32, tag="u1ks")
                            nc.vector.tensor_copy(u1ks, uu[:, :, 0:R])
                            nc.vector.tensor_mul(phik[:, sl, :], u1ks,
                                                 uu[:, :, R : 2 * R])
                            nc.scalar.activation(phik[:, sl, :],
                                                 phik[:, sl, :], AF.Tanh,
                                                 scale=1.0 / SQRT_R)
                            nc.scalar.copy(vau[:, sl, 0:R], vp)

            # ---- tanh (in place, strict fp32), then phi_q hi/lo split
            phqh = perb.tile([R, L], FP16, tag="phqh")
            phql = perb.tile([R, L], FP16, tag="phql")
            for hh in range(2):
                sl = slice(hh * (L // 2), (hh + 1) * (L // 2))
                nc.scalar.activation(phiq[:, sl], phiq[:, sl], AF.Tanh,
                                     scale=1.0 / SQRT_R)
                nc.gpsimd.tensor_copy(phqh[:, sl], phiq[:, sl])
                nc.gpsimd.tensor_sub(phql[:, sl], phiq[:, sl], phqh[:, sl])

            # ---- M~' = [phi_k^T v | s_k]  (strict fp32, accumulated over m)
            mps = psM.tile([128, R + 1], F32, tag="psM")
            for m in range(NT):
                nc.tensor.matmul(mps, lhsT=phik[:, m, :], rhs=vau[:, m, :],
                                 start=(m == 0), stop=(m == NT - 1))
            m_sb = small.tile([128, R + 1], F32, tag="msb")
            nc.vector.tensor_copy(m_sb, mps)
            m_r = small.tile([128, R], FP16, tag="mr")
            nc.scalar.copy(m_r, m_sb[:, 0:R])
            # s_k broadcast to all 128 partitions, then hi/lo split
            s_rep = small.tile([128, 128], F32, tag="srep")
            nc.scalar.activation(s_rep, m_sb[:, 0:R], AF.Identity,
                                 bias=m_sb[:, R : R + 1], scale=0.0)
            srh = small.tile([128, 128], FP16, tag="srh")
            nc.scalar.copy(srh, s_rep)
            srl = small.tile([128, 128], FP16, tag="srl")
            nc.gpsimd.tensor_sub(srl, s_rep, srh)


            # ---- fused tail per chunk: numT -> den (3-pass) -> recip ->
            # proj -> divide -> ship, alternating the two HWDGE rings
            for c in range(NCH):
                sl = slice(c * 512, (c + 1) * 512)
                ntp = psA.tile([128, 512], F32, tag="psA")
                nc.tensor.matmul(ntp, lhsT=m_r, rhs=phqh[:, sl],
                                 start=True, stop=True)
                numt = work.tile([D, 512], F32R, tag="numt")
                nc.scalar.copy(numt, ntp)
                bcp = psA.tile([128, 512], F32, tag="psA")
                nc.tensor.matmul(bcp, lhsT=srh, rhs=phqh[:, sl],
                                 start=True, stop=False)
                nc.tensor.matmul(bcp, lhsT=srh, rhs=phql[:, sl],
                                 start=False, stop=False)
                nc.tensor.matmul(bcp, lhsT=srl, rhs=phqh[:, sl],
                                 start=False, stop=True)
                bcs = work.tile([128, 512], F32, tag="bcs")
                nc.vector.reciprocal(bcs, bcp)
                ptp = psA.tile([128, 512], F32, tag="psA")
                nc.tensor.matmul(ptp, lhsT=pw_f, rhs=numt,
                                 start=True, stop=True)
                ostage = work.tile([D, 512], F32, tag="ost")
                nc.vector.tensor_mul(ostage, ptp, bcs)
                eng = nc.scalar if c % 2 == 0 else nc.sync
                eng.dma_start(out=out_d[b, :, sl], in_=ostage)
    nc.compile()
    return nc


def _split16(a):
    # exact fp16 hi/lo split: a ~ hi + lo with ~2^-22 residual
    a = np.ascontiguousarray(a, np.float32)
    hi = a.astype(np.float16)
    lo = (a - hi.astype(np.float32)).astype(np.float16)
    return hi, lo


def _prep_in_maps(inputs):
    def f32(a):
        return np.ascontiguousarray(np.asarray(a), dtype=np.float32)

    x = f32(inputs["x"])                     # [B, D, L]
    q_w = f32(inputs["q_w"]).reshape(H, D, D, KS)
    k_w = f32(inputs["k_w"]).reshape(H, D, D, KS)
    v_w = f32(inputs["v_w"]).reshape(H, D, D)
    q_b = f32(inputs["q_b"]).reshape(H, D)
    k_b = f32(inputs["k_b"]).reshape(H, D)
    proj_w = f32(inputs["proj_w"])           # [D, H*D]
    gq = float(np.asarray(inputs["gamma_q"]).reshape(-1)[0])
    bq = float(np.asarray(inputs["beta_q"]).reshape(-1)[0])
    gk = float(np.asarray(inputs["gamma_k"]).reshape(-1)[0])
    bk = float(np.asarray(inputs["beta_k"]).reshape(-1)[0])

    xp = np.zeros((D, B, LP), np.float32)
    xp[:, :, KS - 1 :] = x.transpose(1, 0, 2)
    xp_h, xp_l = _split16(xp)
    g_host = np.stack([f32(inputs["g1_q"]), f32(inputs["g2_q"]),
                       f32(inputs["g1_k"]), f32(inputs["g2_k"])], axis=1)
    g_h, g_l = _split16(g_host)

    import ml_dtypes
    in_maps = []
    for h in range(H):
        hblob = np.empty((D, HBLOB_W), np.float16)
        qkw = np.empty((D, 2, KS, D), np.float32)
        qkw[:, 0] = (gq * q_w[h]).transpose(1, 2, 0)  # [c, t, d]
        qkw[:, 1] = (gk * k_w[h]).transpose(1, 2, 0)
        qkw_h, qkw_l = _split16(qkw)
        hblob[:, HOFF_QKWH:HOFF_QKWL] = qkw_h.reshape(D, 2 * KS * D)
        hblob[:, HOFF_QKWL:HOFF_GH] = qkw_l.reshape(D, 2 * KS * D)
        hblob[:, HOFF_GH:HOFF_GL] = g_h.reshape(D, 4 * R)
        hblob[:, HOFF_GL:HOFF_XH0] = g_l.reshape(D, 4 * R)
        hblob[:, HOFF_XH0 : HOFF_XH0 + LP] = xp_h[:, 0]
        hblob[:, HOFF_XL0 : HOFF_XL0 + LP] = xp_l[:, 0]
        hblob[:, HW_W : HW_W + LP] = xp_h[:, 1]
        hblob[:, HW_W + LP : HW_W + 2 * LP] = xp_l[:, 1]
        blob = np.empty((D, BLOB_W), np.float32)
        blob[:, OFF_PW : OFF_PW + D] = proj_w[:, h * D : (h + 1) * D].T
        blob[:, OFF_QKB] = gq * q_b[h] + bq
        blob[:, OFF_QKB + 1] = gk * k_b[h] + bk
        blob[:, OFF_G : OFF_G + 4 * R] = g_host.reshape(D, 4 * R)
        bblob = np.empty((D, BBLOB_W), ml_dtypes.bfloat16)
        bblob[:, BOFF_VW : BOFF_VW + D] = v_w[h].T.astype(ml_dtypes.bfloat16)
        bblob[:, BOFF_X0 : BOFF_X0 + LP] = xp[:, 0].astype(ml_dtypes.bfloat16)
        bblob[:, BOFF_X0 + LP : BOFF_X0 + 2 * LP] = (
            xp[:, 1].astype(ml_dtypes.bfloat16))
        in_maps.append(dict(hblob=hblob, blob=blob, bblob=bblob))
    return in_maps


def kernel(**inputs):
    global _built_nc, last_results
    if _built_nc is None:
        _built_nc = _build()
    in_maps = _prep_in_maps(inputs)
    res = run_bass_kernel_spmd(_built_nc, in_maps, list(range(H)))
    last_results = res
    parts = np.stack([res.results[c]["outp"] for c in range(H)])  # [H, B, D, L]
    out = parts.sum(axis=0, dtype=np.float32).transpose(0, 2, 1)  # [B, L, D]
    out = np.ascontiguousarray(out)
    out += np.asarray(inputs["proj_b"], np.float32)[None, None, :]
    return out.astype(np.float32)


# revision 34
# speedup vs baseline: 1.0091x; 1.0091x over previous
"""Conv1D-MHSA (sketched linear attention) Trainium2 kernel.

Math: the reference computes, per (batch b, head h):
    q = conv1d_K3(x_pad, q_w) + q_b ; k likewise ; v = conv1d_K1(x, v_w)
    phi_q = sqrt(R) * tanh((q^T g1_q) * (q^T g2_q) / sqrt(R))  (phi_k likewise)
    scores = phi_q @ phi_k^T                     [L, L]
    o = (scores / (rowsum(scores) + 1e-6)) @ v   [L, D]
    out = concat_h(o) @ proj_w^T + proj_b
There is no softmax, so the L x L matrix is never needed:
    o = diag(1/(phi_q @ s_k)) . phi_q @ (phi_k^T v),   s_k = colsum(phi_k)
(The sqrt(R) scales cancel; eps is ~5e5x below min |den| and is dropped.
The projection commutes with the row division: project first, divide last.)

Precision scheme (measured): the denominator path amplifies operand rounding
~4000x (near-zero rows of den), so everything feeding phi_q / s_k needs
fp32-level operands.  Strict fp32 matmuls cost 4 cycles/row on the PE, but
fp32r costs 1 cycle/row (free dim >= 256) at ~13-bit operand precision.  So
sensitive operands are split EXACTLY as hi (12-bit mantissa, survives f32r
storage unchanged) + lo (fp32 residual, f32r-stored), and each sensitive
matmul runs 3 fp32r passes (hi*hi + hi*lo + lo*hi, PSUM-accumulated):
3 cycles/row with ~2^-25 composite error.  Weights/x/g split on the host;
q/k and phi_q split on device (ACT copy rounds to the f32r grid, Pool
subtracts the residual).  M~ stays strict fp32 (129-wide rhs would not hit
the fast f32r rate anyway).  The numerator/projection path is tolerant and
runs single-pass f32r (better than the old bf16).  v-conv stays bf16.

Sharding: head-parallel over 8 cores (head h -> core h, both batches).  Each
core returns a partial projection in [j, l] layout [B, D, L]; the host sums
the 8 partials, transposes to [B, L, D] and adds proj_b.  gamma/beta affine
and conv biases are folded into weights on the host.
"""

import numpy as np
from contextlib import ExitStack

import concourse.bacc as bacc
import concourse.mybir as mybir
import concourse.tile as tile
from concourse.bass_utils import run_bass_kernel_spmd

F32 = mybir.dt.float32
F32R = mybir.dt.float32r
FP16 = mybir.dt.float16
BF16 = mybir.dt.bfloat16
AF = mybir.ActivationFunctionType

B = 2          # batch
D = 128        # per-head dim (= partition size)
L = 2048       # sequence length
H = 8          # heads == cores
R = 128        # sketch dim
KS = 3         # conv kernel size
LP = L + KS - 1
NCH = L // 512   # 4 big chunks
NT = L // 128    # 16 tiles
NG = 2           # m-tiles per k/v evacuation group (double-buffered PSUM)
SQRT_R = float(np.sqrt(R))

# fp16 blob: hi/lo split weights + x (free-dim offsets into [128, HBLOB_W])
HOFF_QKWH = 0                        # q/k conv weights hi [2, 3, 128] -> 768
HOFF_QKWL = HOFF_QKWH + 2 * KS * D   # lo -> 1536
HOFF_GH = HOFF_QKWL + 2 * KS * D     # sketch mats hi [4, 128] -> 2048
HOFF_GL = HOFF_GH + 4 * R            # lo -> 2560
HOFF_XH0 = HOFF_GL + 4 * R           # xpad batch 0 hi [2050]
HOFF_XL0 = HOFF_XH0 + LP             # lo
HW_W = HOFF_XL0 + LP                 # 6660 (end of batch-0 block)
HBLOB_W = HW_W + 2 * LP              # x1 hi/lo appended
# fp32 blob: [pw (128) | qk biases (2) | strict g (4x128)]
OFF_PW = 0
OFF_QKB = OFF_PW + D
OFF_G = OFF_QKB + 2
BLOB_W = OFF_G + 4 * R
# bf16 blob: [vw (128) | x0 (2050) | x1 (2050)] = 4228
BOFF_VW = 0
BOFF_X0 = BOFF_VW + D
BBLOB_W = BOFF_X0 + 2 * LP

_built_nc = None
last_results = None


def _build():
    nc = bacc.Bacc(None, target_bir_lowering=False)
    hblob_d = nc.declare_dram_parameter("hblob", [D, HBLOB_W], FP16, isOutput=False)
    blob_d = nc.declare_dram_parameter("blob", [D, BLOB_W], F32, isOutput=False)
    bblob_d = nc.declare_dram_parameter("bblob", [D, BBLOB_W], BF16, isOutput=False)
    out_d = nc.declare_dram_parameter("outp", [B, D, L], F32, isOutput=True)

    def rr(ap):
        return ap.bitcast(F32R)

    with ExitStack() as ctx:
        tc = ctx.enter_context(tile.TileContext(nc))
        consts = ctx.enter_context(tc.tile_pool(name="consts", bufs=1))
        perb = ctx.enter_context(tc.tile_pool(name="perb", bufs=2))
        work = ctx.enter_context(tc.tile_pool(name="work", bufs=4))
        small = ctx.enter_context(tc.tile_pool(name="small", bufs=3))
        psA = ctx.enter_context(tc.tile_pool(name="psA", bufs=4, space="PSUM"))
        psK = ctx.enter_context(tc.tile_pool(name="psK", bufs=2, space="PSUM"))
        psV = ctx.enter_context(tc.tile_pool(name="psV", bufs=1, space="PSUM"))
        psM = ctx.enter_context(tc.tile_pool(name="psM", bufs=1, space="PSUM"))

        # input DMAs spread over three rings: weights via SWDGE (lowest
        # first-byte latency), x0 hi/lo via the SP HWDGE ring, x1 + bf16
        # blob via the ACT HWDGE ring — conv can start early
        wth = consts.tile([D, HOFF_XH0], FP16, tag="wth")
        nc.gpsimd.dma_start(out=wth[:, 0:HOFF_GH], in_=hblob_d[:, 0:HOFF_GH])
        nc.gpsimd.dma_start(out=wth[:, HOFF_GH:HOFF_XH0],
                            in_=hblob_d[:, HOFF_GH:HOFF_XH0])
        pwt = consts.tile([D, D], F32R, tag="pwt")
        nc.gpsimd.dma_start(out=pwt, in_=rr(blob_d[:, OFF_PW : OFF_PW + D]))
        qkbt = consts.tile([D, 2], F32, tag="qkbt")
        nc.gpsimd.dma_start(out=qkbt, in_=blob_d[:, OFF_QKB : OFF_QKB + 2])
        gt = consts.tile([D, 4 * R], F32, tag="gt")
        nc.gpsimd.dma_start(out=gt, in_=blob_d[:, OFF_G : OFF_G + 4 * R])
        x0h = consts.tile([D, LP], FP16, tag="x0h")
        x0l = consts.tile([D, LP], FP16, tag="x0l")
        for s, e in ((0, 1025), (1025, LP)):
            nc.sync.dma_start(out=x0h[:, s:e],
                              in_=hblob_d[:, HOFF_XH0 + s : HOFF_XH0 + e])
            nc.sync.dma_start(out=x0l[:, s:e],
                              in_=hblob_d[:, HOFF_XL0 + s : HOFF_XL0 + e])
        x1h = consts.tile([D, LP], FP16, tag="x1h")
        x1l = consts.tile([D, LP], FP16, tag="x1l")
        nc.scalar.dma_start(out=x1h, in_=hblob_d[:, HW_W : HW_W + LP])
        nc.scalar.dma_start(out=x1l, in_=hblob_d[:, HW_W + LP : HW_W + 2 * LP])
        bb = consts.tile([D, BBLOB_W], BF16, tag="bb")
        nc.scalar.dma_start(out=bb, in_=bblob_d[:])

        qkwh = wth[:, HOFF_QKWH:HOFF_QKWL].rearrange("p (a t d) -> p a t d",
                                                     a=2, t=KS)
        qkwl = wth[:, HOFF_QKWL:HOFF_GH].rearrange("p (a t d) -> p a t d",
                                                   a=2, t=KS)
        g_s = gt.rearrange("p (a r) -> p a r", a=4)
        pw_f = pwt
        xh = [x0h, x1h]
        xl = [x0l, x1l]
        vw_b = bb[:, BOFF_VW : BOFF_VW + D]
        xb = [bb[:, BOFF_X0 + b * LP : BOFF_X0 + (b + 1) * LP] for b in range(B)]

        for b in range(B):
            phiq = perb.tile([R, L], F32, tag="phiq")
            phik = perb.tile([128, NT, R], F32, tag="phik")
            vau = perb.tile([128, NT, R + 1], F32, tag="vau")
            nc.vector.memset(vau[:, :, R], 1.0)
            g12k = g_s[:, 2:4, :].rearrange("p a r -> p (a r)")

            # ---- conv chunks (9-pass split f32r), then per-chunk sketches
            for p in range(2):
                for c in range(NCH):
                    ps = psA.tile([128, 512], F32, tag="psA")
                    mm = []
                    for t in range(KS):
                        xsl = slice(c * 512 + t, c * 512 + t + 512)
                        mm += [(qkwh[:, p, t, :], xh[b][:, xsl]),
                               (qkwl[:, p, t, :], xh[b][:, xsl])]
                    for t in range(KS):
                        xsl = slice(c * 512 + t, c * 512 + t + 512)
                        mm += [(qkwh[:, p, t, :], xl[b][:, xsl])]
                    for i, (w_, x_) in enumerate(mm):
                        nc.tensor.matmul(ps, lhsT=w_, rhs=x_, start=(i == 0),
                                         stop=(i == len(mm) - 1))
                    qc = work.tile([D, 512], F32, tag="qc")
                    nc.scalar.add(qc, ps, qkbt[:, p : p + 1])
                    if p == 0:
                        u1 = psA.tile([128, 512], F32, tag="psA")
                        u2 = psA.tile([128, 512], F32, tag="psA")
                        for ui, u in ((0, u1), (1, u2)):
                            nc.tensor.matmul(u, lhsT=g_s[:, ui, :], rhs=qc,
                                             start=True, stop=True)
                        u1s = work.tile([128, 512], F32, tag="u1s")
                        nc.vector.tensor_copy(u1s, u1)
                        nc.vector.tensor_mul(phiq[:, c * 512 : (c + 1) * 512],
                                             u1s, u2)
                    else:
                        for mg in range(512 // (NG * 128)):
                            uu = psK.tile([128, NG, 2 * R], F32, tag="uu")
                            vp = psV.tile([128, NG, R], F32, tag="vp")
                            for j in range(NG):
                                off = (mg * NG + j) * 128
                                mt = c * 4 + mg * NG + j
                                kl = qc[:, off : off + 128]
                                nc.tensor.matmul(uu[:, j, :], lhsT=kl,
                                                 rhs=g12k, start=True, stop=True)
                                nc.tensor.matmul(
                                    vp[:, j, :],
                                    lhsT=xb[b][:, KS - 1 + mt * 128 :
                                               KS - 1 + (mt + 1) * 128],
                                    rhs=vw_b, start=True, stop=True)
                            sl = slice(c * 4 + mg * NG, c * 4 + (mg + 1) * NG)
                            u1ks = work.tile([128, NG, R], F32, tag="u1ks")
                            nc.vector.tensor_copy(u1ks, uu[:, :, 0:R])
                            nc.vector.tensor_mul(phik[:, sl, :], u1ks,
                                                 uu[:, :, R : 2 * R])
                            nc.scalar.activation(phik[:, sl, :],
                                                 phik[:, sl, :], AF.Tanh,
                                                 scale=1.0 / SQRT_R)
                            nc.scalar.copy(vau[:, sl, 0:R], vp)

            # ---- tanh (in place, strict fp32), then phi_q hi/lo split
            phqh = perb.tile([R, L], FP16, tag="phqh")
            phql = perb.tile([R, L], FP16, tag="phql")
            for hh in range(2):
                sl = slice(hh * (L // 2), (hh + 1) * (L // 2))
                nc.scalar.activation(phiq[:, sl], phiq[:, sl], AF.Tanh,
                                     scale=1.0 / SQRT_R)
                nc.gpsimd.tensor_copy(phqh[:, sl], phiq[:, sl])
                nc.gpsimd.tensor_sub(phql[:, sl], phiq[:, sl], phqh[:, sl])

            # ---- M~' = [phi_k^T v | s_k]  (strict fp32, accumulated over m)
            mps = psM.tile([128, R + 1], F32, tag="psM")
            for m in range(NT):
                nc.tensor.matmul(mps, lhsT=phik[:, m, :], rhs=vau[:, m, :],
                                 start=(m == 0), stop=(m == NT - 1))
            m_sb = small.tile([128, R + 1], F32, tag="msb")
            nc.vector.tensor_copy(m_sb, mps)
            m_r = small.tile([128, R], FP16, tag="mr")
            nc.scalar.copy(m_r, m_sb[:, 0:R])
            # s_k broadcast to all 128 partitions, then hi/lo split
            s_rep = small.tile([128, 128], F32, tag="srep")
            nc.scalar.activation(s_rep, m_sb[:, 0:R], AF.Identity,
                                 bias=m_sb[:, R : R + 1], scale=0.0)
            srh = small.tile([128, 128], FP16, tag="srh")
            nc.scalar.copy(srh, s_rep)
            srl = small.tile([128, 128], FP16, tag="srl")
            nc.gpsimd.tensor_sub(srl, s_rep, srh)


            # ---- fused tail per chunk: numT -> den (3-pass) -> recip ->
            # proj -> divide -> ship, alternating the two HWDGE rings
            for c in range(NCH):
                sl = slice(c * 512, (c + 1) * 512)
                ntp = psA.tile([128, 512], F32, tag="psA")
                nc.tensor.matmul(ntp, lhsT=m_r, rhs=phqh[:, sl],
                                 start=True, stop=True)
                numt = work.tile([D, 512], F32R, tag="numt")
                nc.scalar.copy(numt, ntp)
                bcp = psA.tile([128, 512], F32, tag="psA")
                nc.tensor.matmul(bcp, lhsT=srh, rhs=phqh[:, sl],
                                 start=True, stop=False)
                nc.tensor.matmul(bcp, lhsT=srh, rhs=phql[:, sl],
                                 start=False, stop=False)
                nc.tensor.matmul(bcp, lhsT=srl, rhs=phqh[:, sl],
                                 start=False, stop=True)
                bcs = work.tile([128, 512], F32, tag="bcs")
                nc.vector.reciprocal(bcs, bcp)
                ptp = psA.tile([128, 512], F32, tag="psA")
                nc.tensor.matmul(ptp, lhsT=pw_f, rhs=numt,
                                 start=True, stop=True)
                ostage = work.tile([D, 512], F32, tag="ost")
                nc.vector.tensor_mul(ostage, ptp, bcs)
                eng = nc.scalar if c % 2 == 0 else nc.sync
                eng.dma_start(out=out_d[b, :, sl], in_=ostage)
    nc.compile()
    return nc


def _split16(a):
    # exact fp16 hi/lo split: a ~ hi + lo with ~2^-22 residual
    a = np.ascontiguousarray(a, np.float32)
    hi = a.astype(np.float16)
    lo = (a - hi.astype(np.float32)).astype(np.float16)
    return hi, lo


def _prep_in_maps(inputs):
    def f32(a):
        return np.ascontiguousarray(np.asarray(a), dtype=np.float32)

    x = f32(inputs["x"])                     # [B, D, L]
    q_w = f32(inputs["q_w"]).reshape(H, D, D, KS)
    k_w = f32(inputs["k_w"]).reshape(H, D, D, KS)
    v_w = f32(inputs["v_w"]).reshape(H, D, D)
    q_b = f32(inputs["q_b"]).reshape(H, D)
    k_b = f32(inputs["k_b"]).reshape(H, D)
    proj_w = f32(inputs["proj_w"])           # [D, H*D]
    gq = float(np.asarray(inputs["gamma_q"]).reshape(-1)[0])
    bq = float(np.asarray(inputs["beta_q"]).reshape(-1)[0])
    gk = float(np.asarray(inputs["gamma_k"]).reshape(-1)[0])
    bk = float(np.asarray(inputs["beta_k"]).reshape(-1)[0])

    xp = np.zeros((D, B, LP), np.float32)
    xp[:, :, KS - 1 :] = x.transpose(1, 0, 2)
    xp_h, xp_l = _split16(xp)
    g_host = np.stack([f32(inputs["g1_q"]), f32(inputs["g2_q"]),
                       f32(inputs["g1_k"]), f32(inputs["g2_k"])], axis=1)
    g_h, g_l = _split16(g_host)

    import ml_dtypes
    in_maps = []
    for h in range(H):
        hblob = np.empty((D, HBLOB_W), np.float16)
        qkw = np.empty((D, 2, KS, D), np.float32)
        qkw[:, 0] = (gq * q_w[h]).transpose(1, 2, 0)  # [c, t, d]
        qkw[:, 1] = (gk * k_w[h]).transpose(1, 2, 0)
        qkw_h, qkw_l = _split16(qkw)
        hblob[:, HOFF_QKWH:HOFF_QKWL] = qkw_h.reshape(D, 2 * KS * D)
        hblob[:, HOFF_QKWL:HOFF_GH] = qkw_l.reshape(D, 2 * KS * D)
        hblob[:, HOFF_GH:HOFF_GL] = g_h.reshape(D, 4 * R)
        hblob[:, HOFF_GL:HOFF_XH0] = g_l.reshape(D, 4 * R)
        hblob[:, HOFF_XH0 : HOFF_XH0 + LP] = xp_h[:, 0]
        hblob[:, HOFF_XL0 : HOFF_XL0 + LP] = xp_l[:, 0]
        hblob[:, HW_W : HW_W + LP] = xp_h[:, 1]
        hblob[:, HW_W + LP : HW_W + 2 * LP] = xp_l[:, 1]
        blob = np.empty((D, BLOB_W), np.float32)
        blob[:, OFF_PW : OFF_PW + D] = proj_w[:, h * D : (h + 1) * D].T
        blob[:, OFF_QKB] = gq * q_b[h] + bq
        blob[:, OFF_QKB + 1] = gk * k_b[h] + bk
        blob[:, OFF_G : OFF_G + 4 * R] = g_host.reshape(D, 4 * R)
        bblob = np.empty((D, BBLOB_W), ml_dtypes.bfloat16)
        bblob[:, BOFF_VW : BOFF_VW + D] = v_w[h].T.astype(ml_dtypes.bfloat16)
        bblob[:, BOFF_X0 : BOFF_X0 + LP] = xp[:, 0].astype(ml_dtypes.bfloat16)
        bblob[:, BOFF_X0 + LP : BOFF_X0 + 2 * LP] = (
            xp[:, 1].astype(ml_dtypes.bfloat16))
        in_maps.append(dict(hblob=hblob, blob=blob, bblob=bblob))
    return in_maps


def kernel(**inputs):
    global _built_nc, last_results
    if _built_nc is None:
        _built_nc = _build()
    in_maps = _prep_in_maps(inputs)
    res = run_bass_kernel_spmd(_built_nc, in_maps, list(range(H)))
    last_results = res
    parts = np.stack([res.results[c]["outp"] for c in range(H)])  # [H, B, D, L]
    out = parts.sum(axis=0, dtype=np.float32).transpose(0, 2, 1)  # [B, L, D]
    out = np.ascontiguousarray(out)
    out += np.asarray(inputs["proj_b"], np.float32)[None, None, :]
    return out.astype(np.float32)
